# revision 2
# baseline (speedup 1.0000x reference)
"""AdaptiveTokenSampling on 8 TRN2 NeuronCores (Bass/Tile, data-parallel over batch).

kernel(**inputs) takes the FULL inputs and returns the FULL outputs
(new_attn f32, new_mask bool, uniq_ids int32), sharding batch B=16 as 2
batches per core. All per-batch work (scoring, CDF, inverse-CDF sampling,
dedup, gather) runs on-device; the host only shards inputs and concatenates
shard outputs.

Numerical strategy: the sampled token ids are discrete argmin decisions, so
the on-device CDF must match the float32 reference closely. All real-valued
arithmetic runs on DVE/ACT in IEEE f32 (sequential prefix scan for the
cumsum); the TensorEngine only ever sees 0/1-integer data (exact under the
fp32 hi/lo matmul split); sqrt is computed as v*rsqrt(v) with a table seed
polished by two Newton iterations. The nearest-CDF-index argmin is computed
as a midpoint-count: sampled[s]-1 = #{t: cdf[t]+cdf[t+1] < 2*step_s*total'},
which matches jnp.argmin's first-index tie-breaking.
"""

from contextlib import ExitStack

import numpy as np

import concourse.bacc as bacc
import concourse.bass as bass
import concourse.mybir as mybir
import concourse.tile as tile
from concourse.bass_utils import run_bass_kernel_spmd

F32 = mybir.dt.float32
I32 = mybir.dt.int32
I16 = mybir.dt.int16

N_CORES = 8
B = 16
B_LOC = B // N_CORES
H = 16
N = 1025
T = 1024
D = 64
K = 256
S = K - 1
ROWS = B_LOC * H * N

AX = mybir.AxisListType.X
OP = mybir.AluOpType


def _build_core_graph():
    """One NeuronCore's program: B_LOC batches, all heads."""
    nc = bacc.Bacc("TRN2", target_bir_lowering=False, debug=False)

    x_ap = nc.dram_tensor("x", [B_LOC, H, N, D], F32, kind="ExternalInput").ap()
    attn_ap = nc.dram_tensor("attn", [ROWS, N], F32, kind="ExternalInput").ap()
    steps2_ap = nc.dram_tensor("steps2", [128, S], F32, kind="ExternalInput").ap()
    out_attn = nc.dram_tensor(
        "out_attn", [B_LOC, H, K, N], F32, kind="ExternalOutput"
    ).ap()
    out_ids = nc.dram_tensor("out_ids", [B_LOC, K], I32, kind="ExternalOutput").ap()

    attn3 = attn_ap.rearrange("(r n) m -> r n m", n=N)

    with tile.TileContext(nc) as tc, ExitStack() as ctx:
        cp = ctx.enter_context(tc.tile_pool(name="const", bufs=1))
        sb = ctx.enter_context(tc.tile_pool(name="sb", bufs=2))
        xp = ctx.enter_context(tc.tile_pool(name="xp", bufs=3))
        gp = ctx.enter_context(tc.tile_pool(name="gp", bufs=6))
        pp = ctx.enter_context(tc.tile_pool(name="pp", bufs=2, space="PSUM"))

        steps2 = cp.tile([128, S], F32)
        nc.sync.dma_start(out=steps2[:], in_=steps2_ap[:, :])
        ones_col = cp.tile([128, 1], F32)
        nc.vector.memset(ones_col[:], 1.0)

        for b in range(B_LOC):
            # cls attention row (CLS -> tokens) for every head
            cls_rows = sb.tile([16, T], F32, tag="cls_rows")
            nc.sync.dma_start(out=cls_rows[:], in_=attn3[b * H:(b + 1) * H, 0, 1:])

            # squared value norms, token t=8p+j on partition p, head h on free
            vnsq = sb.tile([128, H * 8], F32, tag="vnsq")
            for h in range(H):
                xx = xp.tile([128, 512], F32, tag="xx")
                nc.sync.dma_start(
                    out=xx[:],
                    in_=x_ap[b, h, 1:, :].rearrange("(p j) d -> p (j d)", p=128),
                )
                sq = xp.tile([128, 512], F32, tag="sq")
                nc.vector.tensor_mul(out=sq[:], in0=xx[:], in1=xx[:])
                nc.vector.tensor_reduce(
                    out=vnsq[:, h * 8:(h + 1) * 8],
                    in_=sq[:].rearrange("p (j d) -> p j d", d=D),
                    axis=AX,
                    op=OP.add,
                )

            # vn = vnsq * rsqrt(vnsq); seed 1/ACT-sqrt + 2 Newton iterations
            r = sb.tile([128, H * 8], F32, tag="r")
            nc.scalar.activation(
                out=r[:], in_=vnsq[:], func=mybir.ActivationFunctionType.Sqrt
            )
            nc.vector.reciprocal(out=r[:], in_=r[:])
            t1 = sb.tile([128, H * 8], F32, tag="t1")
            for _ in range(2):
                nc.vector.tensor_mul(out=t1[:], in0=r[:], in1=r[:])
                nc.vector.tensor_mul(out=t1[:], in0=t1[:], in1=vnsq[:])
                nc.vector.tensor_scalar(
                    out=t1[:], in0=t1[:], scalar1=-0.5, scalar2=1.5,
                    op0=OP.mult, op1=OP.add,
                )
                nc.vector.tensor_mul(out=r[:], in0=r[:], in1=t1[:])
            vn = sb.tile([128, H * 8], F32, tag="vn")
            nc.vector.tensor_mul(out=vn[:], in0=vnsq[:], in1=r[:])

            # cls rows reordered to match the (token-on-partition, head) layout
            cls_all = sb.tile([128, H * 8], F32, tag="cls_all")
            for h in range(H):
                nc.scalar.dma_start(
                    out=cls_all[:, h * 8:(h + 1) * 8],
                    in_=cls_rows[h:h + 1, :].rearrange("a (p j) -> a p j", j=8),
                )

            # head-summed significance
            prod = sb.tile([128, H * 8], F32, tag="prod")
            nc.vector.tensor_mul(out=prod[:], in0=vn[:], in1=cls_all[:])
            sig_col = sb.tile([128, 8], F32, tag="sig_col")
            nc.vector.tensor_reduce(
                out=sig_col[:],
                in_=prod[:].rearrange("p (h j) -> p j h", j=8),
                axis=AX,
                op=OP.add,
            )

            # sequential f32 cumulative sum on a single row
            sig_row = sb.tile([1, T], F32, tag="sig_row")
            nc.sync.dma_start(
                out=sig_row[:].rearrange("a (p j) -> a p j", j=8), in_=sig_col[:]
            )
            cdf_row = sb.tile([1, T], F32, tag="cdf_row")
            nc.vector.tensor_tensor_scan(
                out=cdf_row[:], data0=sig_row[:], data1=sig_row[:],
                initial=0.0, op0=OP.add, op1=OP.bypass,
            )
            denom = sb.tile([1, 1], F32, tag="denom")
            nc.vector.tensor_scalar_add(denom[:], cdf_row[0:1, T - 1:T], 1e-6)

            # unnormalized midpoints cdf[t]+cdf[t+1] (sentinel on the last)
            mids_row = sb.tile([1, T], F32, tag="mids_row")
            nc.vector.tensor_add(
                out=mids_row[0:1, 0:T - 1],
                in0=cdf_row[0:1, 0:T - 1],
                in1=cdf_row[0:1, 1:T],
            )
            nc.vector.memset(mids_row[0:1, T - 1:T], 1e30)
            mids_col = sb.tile([128, 8], F32, tag="mids_col")
            nc.sync.dma_start(
                out=mids_col[:], in_=mids_row[:].rearrange("a (p j) -> a p j", j=8)
            )

            # counts[s] = #{t: mid_t < 2*step_s*denom}
            denom_bc = sb.tile([128, 1], F32, tag="denom_bc")
            nc.gpsimd.partition_broadcast(denom_bc[:], denom[:], channels=128)
            qd2 = sb.tile([128, S], F32, tag="qd2")
            nc.vector.tensor_scalar(
                out=qd2[:], in0=steps2[:], scalar1=denom_bc[:, 0:1],
                scalar2=None, op0=OP.mult,
            )
            acc = sb.tile([128, S], F32, tag="acc")
            nc.vector.tensor_scalar(
                out=acc[:], in0=qd2[:], scalar1=mids_col[:, 0:1],
                scalar2=None, op0=OP.is_gt,
            )
            tmp = sb.tile([128, S], F32, tag="tmp")
            for j in range(1, 8):
                nc.vector.tensor_scalar(
                    out=tmp[:], in0=qd2[:], scalar1=mids_col[:, j:j + 1],
                    scalar2=None, op0=OP.is_gt,
                )
                nc.vector.tensor_add(out=acc[:], in0=acc[:], in1=tmp[:])
            ps0 = pp.tile([128, 1], F32, tag="ps0")
            nc.tensor.matmul(
                out=ps0[:], lhsT=acc[:, 0:128], rhs=ones_col[:],
                start=True, stop=True,
            )
            ps1 = pp.tile([128, 1], F32, tag="ps1")
            nc.tensor.matmul(
                out=ps1[0:127, :], lhsT=acc[:, 128:S], rhs=ones_col[:],
                start=True, stop=True,
            )
            counts_sb = sb.tile([128, 2], F32, tag="counts_sb")
            nc.vector.tensor_copy(out=counts_sb[:, 0:1], in_=ps0[:])
            nc.vector.tensor_copy(out=counts_sb[0:127, 1:2], in_=ps1[0:127, :])

            # sampled-1, already sorted (counts are monotone in s)
            srt_row = sb.tile([1, 256], F32, tag="srt_row")
            nc.sync.dma_start(
                out=srt_row[0:1, 0:128].rearrange("a (p j) -> a p j", j=1),
                in_=counts_sb[:, 0:1],
            )
            nc.sync.dma_start(
                out=srt_row[0:1, 128:S].rearrange("a (p j) -> a p j", j=1),
                in_=counts_sb[0:127, 1:2],
            )

            # dedup to sorted-unique with zero padding, CLS id 0 in front:
            # rank = inclusive scan of first-occurrence flags; scatter ids to
            # rank (duplicates -> index -1, dropped by local_scatter)
            first = sb.tile([1, S], F32, tag="first")
            nc.vector.memset(first[0:1, 0:1], 1.0)
            nc.vector.tensor_tensor(
                out=first[0:1, 1:S], in0=srt_row[0:1, 1:S],
                in1=srt_row[0:1, 0:S - 1], op=OP.not_equal,
            )
            cum = sb.tile([1, S], F32, tag="cum")
            nc.vector.tensor_tensor_scan(
                out=cum[:], data0=first[:], data1=first[:],
                initial=0.0, op0=OP.add, op1=OP.bypass,
            )
            posf = sb.tile([1, 256], F32, tag="posf")
            nc.vector.tensor_scalar_add(posf[0:1, 0:S], cum[:], 1.0)
            nc.vector.tensor_mul(out=posf[0:1, 0:S], in0=posf[0:1, 0:S], in1=first[:])
            nc.vector.tensor_scalar_add(posf[0:1, 0:S], posf[0:1, 0:S], -1.0)
            nc.vector.memset(posf[0:1, S:256], -1.0)
            dataf = sb.tile([1, S], F32, tag="dataf")
            nc.vector.tensor_scalar_add(dataf[:], srt_row[0:1, 0:S], 1.0)

            idxs16 = sb.tile([16, 256], I16, tag="idxs16")
            nc.vector.memset(idxs16[:], -1)
            nc.vector.tensor_copy(out=idxs16[0:1, :], in_=posf[:])
            data16 = sb.tile([16, 256], I16, tag="data16")
            nc.vector.memset(data16[:], 0)
            nc.vector.tensor_copy(out=data16[0:1, 0:S], in_=dataf[:])
            usc = sb.tile([16, 256], I16, tag="usc")
            nc.gpsimd.local_scatter(
                out_ap=usc[:], data_ap=data16[:], idxs_ap=idxs16[:],
                channels=16, num_elems=256, num_idxs=256,
            )

            ids32 = sb.tile([1, 256], I32, tag="ids32")
            nc.vector.tensor_copy(out=ids32[:], in_=usc[0:1, :])
            nc.sync.dma_start(out=out_ids[b:b + 1, :], in_=ids32[:])
            idx_col = sb.tile([128, 2], I32, tag="idx_col")
            nc.sync.dma_start(
                out=idx_col[:, 0:1],
                in_=ids32[0:1, 0:128].rearrange("a (p j) -> a p j", j=1),
            )
            nc.sync.dma_start(
                out=idx_col[:, 1:2],
                in_=ids32[0:1, 128:256].rearrange("a (p j) -> a p j", j=1),
            )

            # gather the selected attention rows per head, stream to output
            for h in range(H):
                for c in range(2):
                    g = gp.tile([128, N], F32, tag="g")
                    nc.gpsimd.indirect_dma_start(
                        out=g[:],
                        out_offset=None,
                        in_=attn_ap[:, :],
                        in_offset=bass.IndirectOffsetOnAxis(
                            ap=idx_col[:, c:c + 1], axis=0
                        ),
                        element_offset=(b * H + h) * N * N,
                    )
                    nc.scalar.dma_start(
                        out=out_attn[b, h, c * 128:(c + 1) * 128, :], in_=g[:]
                    )

    nc.compile()
    return nc


_NC_CACHE = None


def _get_graph():
    global _NC_CACHE
    if _NC_CACHE is None:
        _NC_CACHE = _build_core_graph()
    return _NC_CACHE


def kernel(x, attn, mask, sample_count, _profile_out=None):
    x = np.asarray(x)
    attn = np.asarray(attn)
    mask = np.asarray(mask)
    sc = int(np.asarray(sample_count))
    assert x.shape == (B, H, N, D) and attn.shape == (B, H, N, N)
    assert sc == K, f"kernel compiled for sample_count={K}, got {sc}"
    assert bool(np.all(mask)), "kernel assumes an all-True mask (spec fill=ones)"

    steps = ((1.0 + 2.0 * np.arange(S, dtype=np.float32)) / (2.0 * K)).astype(
        np.float32
    )
    steps2 = np.ascontiguousarray(
        np.broadcast_to((2.0 * steps)[None, :], (128, S))
    )

    nc = _get_graph()
    in_maps = []
    for c in range(N_CORES):
        bsl = slice(c * B_LOC, (c + 1) * B_LOC)
        in_maps.append(
            {
                "x": np.ascontiguousarray(x[bsl]).astype(np.float32, copy=False),
                "attn": np.ascontiguousarray(attn[bsl]).reshape(ROWS, N),
                "steps2": steps2,
            }
        )

    want_trace = _profile_out is not None
    try:
        res = run_bass_kernel_spmd(
            nc, in_maps, core_ids=list(range(N_CORES)), trace=want_trace
        )
    except ImportError:
        # NTFF profile hook unavailable in this image; run without tracing.
        res = run_bass_kernel_spmd(
            nc, in_maps, core_ids=list(range(N_CORES)), trace=False
        )
    if want_trace:
        _profile_out["exec_time_ns"] = res.exec_time_ns
        _profile_out["results"] = res

    new_attn = np.concatenate([r["out_attn"] for r in res.results], axis=0)
    uniq_ids = np.concatenate([r["out_ids"] for r in res.results], axis=0).astype(
        np.int32
    )
    new_mask = uniq_ids != 0
    new_mask[:, 0] = True
    return new_attn.astype(np.float32, copy=False), new_mask, uniq_ids
